# revision 13
# baseline (speedup 1.0000x reference)
"""3-level db4 wavelet decomposition (pywt mode='symmetric') on Trainium2.

x: (64, 4096, 128) fp32 -> (low (64,518,128), cD3 (64,518,128),
cD2 (64,1029,128), cD1 (64,2051,128)).

Sharding: pure data parallel over B — 8 batches per core on 8 cores.

Per-core structure: each DWT level is a banded matrix along T; T lives on
the SBUF partition axis. One out-tile = 61 output positions; a single
TensorE matmul per tile computes low+high stacked ([<=128 input rows,
2*61 out cols]), rhs packs F=128 x 4 batches = 512 signals in the free
dim. Each PSUM result gets exactly one full-width copy to an SBUF
staging tile ([61 cA rows ; 61 cD rows] on partitions). Detail rows are
DMA'd to HBM from staging; the next level consumes the cA halves of the
staging tiles *directly* as matmul rhs, accumulating in PSUM over the
2 full + 1 six-row source tiles its 134-row input span straddles
(copy cost on DVE/ACT scales only with the free dim, so avoiding
scatter copies matters more than extra matmul streams).
"""

import sys

import numpy as np

for _p in ("/opt/trn_rl_repo",):
    if _p not in sys.path:
        sys.path.insert(0, _p)

import concourse.bacc as bacc
import concourse.mybir as mybir
import concourse.tile as tile
from concourse.bass_utils import run_bass_kernel_spmd

F32 = mybir.dt.float32
F32R = mybir.dt.float32r

FILT_LEN = 8
O_TILE = 61  # outputs per matmul tile (2*61+6 = 128 input rows)
N_CORES = 8
B, T0, F = 64, 4096, 128
NB = B // N_CORES          # batches per core
GB = 4                     # batches per group (4*128 = 512 = rhs free dim)
GROUPS = NB // GB
NFREE = GB * F             # 512

DEC_LO = np.array([-0.010597401784997278, 0.032883011666982945,
                   0.030841381835986965, -0.18703481171888114,
                   -0.02798376941698385, 0.6308807679295904,
                   0.7148465705525415, 0.23037781330885523], dtype=np.float64)
DEC_HI = np.array([-0.23037781330885523, 0.7148465705525415,
                   -0.6308807679295904, -0.02798376941698385,
                   0.18703481171888114, 0.030841381835986965,
                   -0.032883011666982945, -0.010597401784997278], dtype=np.float64)

T1 = (T0 + FILT_LEN - 1) // 2   # 2051
T2 = (T1 + FILT_LEN - 1) // 2   # 1029
T3 = (T2 + FILT_LEN - 1) // 2   # 518
LEVEL_T = [(T0, T1), (T1, T2), (T2, T3)]

USE_F32R = True  # fp32r matmul: full-rate PE, ~tf32 mantissa; else fp32 exact


def _dwt_matrix(T, filt):
    """(T_out, T): out = M @ sig replicating reference _dwt_step
    (symmetric ext by 7 each side, drop first, stride-2 conv)."""
    T_out = (T + FILT_LEN - 1) // 2
    M = np.zeros((T_out, T), dtype=np.float64)

    def src(i):
        if i < 7:
            return 6 - i
        j = i - 7
        if j < T:
            return j
        return T - 1 - (j - T)

    for p in range(T_out):
        for m in range(FILT_LEN):
            M[p, src(2 * p + 8 - m)] += filt[m]
    return M


def _tiles_for_level(T):
    """[(p0, Oc, a, w)]: out tile rows [p0,p0+Oc) read sig cols [a,a+w)."""
    T_out = (T + FILT_LEN - 1) // 2
    tiles = []
    p0 = 0
    while p0 < T_out:
        Oc = min(O_TILE, T_out - p0)
        lo_i = 2 * p0 + 1
        hi_i = 2 * (p0 + Oc - 1) + 8
        cols = [max(0, lo_i - 7), min(T - 1, hi_i - 7)]
        if lo_i < 7:
            cols.append(6 - lo_i)
        if hi_i >= T + 7:
            cols.append(T - 1 - (hi_i - 7 - T))
            cols.append(T - 1)
        a, b = min(cols), max(cols)
        tiles.append((p0, Oc, a, b - a + 1))
        p0 += Oc
    return tiles


class _WeightPacker:
    def __init__(self):
        self.blocks = []   # (row_offset, blk)
        self.offsets = {}
        self.cols = 0

    def add(self, blk, row_offset=0):
        blk = np.ascontiguousarray(blk, dtype=np.float32)
        key = (row_offset, blk.shape, blk.tobytes())
        if key not in self.offsets:
            self.offsets[key] = self.cols
            self.blocks.append((row_offset, blk))
            self.cols += blk.shape[1]
        return self.offsets[key]

    def pack(self):
        wpack = np.zeros((128, max(self.cols, 1)), dtype=np.float32)
        col = 0
        for ro, b in self.blocks:
            wpack[ro:ro + b.shape[0], col:col + b.shape[1]] = b
            col += b.shape[1]
        return wpack


def _build_plans():
    """Per level, a list of out-tile plans.

    Level 0 plan entries:  (p0, Oc, [("hbm", a, w, woff)])
    Level 1/2 entries:     (p0, Oc, [(j, b, r1, woff)]) — source staging
    tile j of the previous level holds cA rows [61j, 61j+61) on partitions
    [0, 61); the piece reads its partitions [b, r1).  The PE requires the
    base partition to be 32-aligned, so pieces starting mid-tile are
    padded down to partition 0/32 with zero weight rows.
    """
    wp = _WeightPacker()
    plans = []
    for lvl, (Tin, _Tout) in enumerate(LEVEL_T):
        Ml = _dwt_matrix(Tin, DEC_LO)
        Mh = _dwt_matrix(Tin, DEC_HI)
        tiles = _tiles_for_level(Tin)
        plan = []
        for (p0, Oc, a, w) in tiles:
            def wblk(c0, c1):
                return np.concatenate(
                    [Ml[p0:p0 + Oc, c0:c1].T, Mh[p0:p0 + Oc, c0:c1].T],
                    axis=1)

            if lvl == 0:
                pieces = [("hbm", a, w, wp.add(wblk(a, a + w)))]
            else:
                pieces = []
                j0, j1 = a // O_TILE, (a + w - 1) // O_TILE
                for j in range(j0, j1 + 1):
                    lo = max(a, j * O_TILE)
                    hi = min(a + w, (j + 1) * O_TILE)
                    r0, r1 = lo - j * O_TILE, hi - j * O_TILE
                    b = 32 if r0 >= 32 else 0
                    blk = np.zeros((r1 - b, 2 * Oc))
                    blk[r0 - b:] = wblk(lo, hi)
                    pieces.append((j, b, r1, wp.add(blk, row_offset=b)))
            plan.append((p0, Oc, pieces))
        plans.append(plan)
    return wp.pack(), plans


WPACK, PLANS = _build_plans()
WCOLS = WPACK.shape[1]

_CACHE = {}

# data dtype for DRAM tensors and SBUF tiles feeding the PE: declaring them
# fp32r end-to-end satisfies the BIR verifier rule that fp32r matmul operands
# are produced as fp32r (numpy-side both map to float32; PSUM stays fp32)
DT = F32R if USE_F32R else F32


def build_program():
    nc = bacc.Bacc("TRN2", target_bir_lowering=False, debug=False,
                   num_devices=N_CORES)
    x_h = nc.dram_tensor("x", [NB, T0, F], DT, kind="ExternalInput")
    w_h = nc.dram_tensor("w", [128, WCOLS], DT, kind="ExternalInput")
    low_h = nc.dram_tensor("low", [NB, T3, F], DT, kind="ExternalOutput")
    cd3_h = nc.dram_tensor("cd3", [NB, T3, F], DT, kind="ExternalOutput")
    cd2_h = nc.dram_tensor("cd2", [NB, T2, F], DT, kind="ExternalOutput")
    cd1_h = nc.dram_tensor("cd1", [NB, T1, F], DT, kind="ExternalOutput")
    det_h = [cd1_h, cd2_h, cd3_h]

    copy_rr = [0]

    store_rr = [0]

    with tile.TileContext(nc) as tc:
        with (
            tc.tile_pool(name="wp", bufs=1) as wpool,
            tc.tile_pool(name="xin", bufs=8) as xin,
            tc.tile_pool(name="st0", bufs=len(PLANS[0])) as st0p,
            tc.tile_pool(name="st1", bufs=len(PLANS[1])) as st1p,
            tc.tile_pool(name="st2", bufs=len(PLANS[2])) as st2p,
            tc.tile_pool(name="ps", bufs=6, space="PSUM") as psp,
        ):
            stpools = [st0p, st1p, st2p]
            wt = wpool.tile([128, WCOLS], DT)
            nc.sync.dma_start(out=wt[:], in_=w_h[:])

            def copy_stage(dst, src):
                if copy_rr[0] % 2 == 0:
                    nc.vector.tensor_copy(dst, src)
                else:
                    nc.scalar.copy(dst, src)
                copy_rr[0] += 1

            # loads go on SP; stores rotate over the other issue queues so no
            # single sequencer serializes all the DMA traffic
            store_engines = [nc.scalar, nc.gpsimd, nc.sync]

            def store(dst, src):
                eng = store_engines[store_rr[0] % len(store_engines)]
                store_rr[0] += 1
                eng.dma_start(out=dst, in_=src)

            for g in range(GROUPS):
                b0 = GB * g

                def dram_g(h, p0, Oc):
                    return h[b0:b0 + GB, p0:p0 + Oc, :].rearrange(
                        "b p f -> p b f")

                def sb3(ap):
                    return ap.rearrange("p (b f) -> p b f", b=GB)

                prev_stage = None
                for lvl in range(3):
                    plan = PLANS[lvl]
                    stage = [
                        stpools[lvl].tile([128, NFREE], DT,
                                          name=f"st{lvl}_{g}_{i}",
                                          tag=f"st{lvl}")
                        for i in range(len(plan))
                    ]
                    for c, (p0, Oc, pieces) in enumerate(plan):
                        ps = psp.tile([2 * Oc, NFREE], F32,
                                      name=f"ps_{lvl}_{c}", tag="ps")
                        for i, piece in enumerate(pieces):
                            first, last = i == 0, i == len(pieces) - 1
                            if piece[0] == "hbm":
                                _, a, w, off = piece
                                rhs = xin.tile([128, NFREE], DT,
                                               name=f"x_{c}", tag="xin")
                                nc.sync.dma_start(out=sb3(rhs[:w]),
                                                  in_=dram_g(x_h, a, w))
                                src = rhs[:w]
                                lhsT = wt[:w, off:off + 2 * Oc]
                            else:
                                j, pb, r1, off = piece
                                src = prev_stage[j][pb:r1]
                                lhsT = wt[pb:r1, off:off + 2 * Oc]
                            nc.tensor.matmul(
                                ps[:], lhsT, src,
                                start=first, stop=last)
                        st = stage[c]
                        copy_stage(st[:2 * Oc], ps[:])
                        store(dram_g(det_h[lvl], p0, Oc), sb3(st[Oc:2 * Oc]))
                        if lvl == 2:
                            store(dram_g(low_h, p0, Oc), sb3(st[:Oc]))
                    prev_stage = stage

    nc.compile()
    return nc


def kernel(x: np.ndarray):
    x = np.ascontiguousarray(x, dtype=np.float32)
    assert x.shape == (B, T0, F), x.shape

    if "nc" not in _CACHE:
        _CACHE["nc"] = build_program()
    nc = _CACHE["nc"]

    in_maps = [
        {"x": x[c * NB:(c + 1) * NB], "w": WPACK} for c in range(N_CORES)
    ]
    res = run_bass_kernel_spmd(nc, in_maps, core_ids=list(range(N_CORES)))
    low = np.concatenate([r["low"] for r in res.results], axis=0)
    cd3 = np.concatenate([r["cd3"] for r in res.results], axis=0)
    cd2 = np.concatenate([r["cd2"] for r in res.results], axis=0)
    cd1 = np.concatenate([r["cd1"] for r in res.results], axis=0)
    return (low, cd3, cd2, cd1)
